# revision 7
# baseline (speedup 1.0000x reference)
"""Trainium2 Bass kernel for nn_MixtureAttention (B=2, S=2048, D=1024, H=16).

Sharding: 8 cores = 2 batches x 4 head-groups (4 heads each, Megatron-style
tensor parallel). Each core computes, for its batch b and feature slice
fsl (256 features = 4 heads):

  Q^T  = (Wq[fsl] . x^T) + (pe Wq^T + bq)^T   [256, 2048]  (float32r matmuls)
  K^T  likewise; V = x Wv^T + (pe Wv^T + bv)  [2048, 256] natural layout
  S^T  = per-head scores in [k_tok, q] PSUM tiles (head pair row-packed)
  P^T  = exp(S^T / 32)            (no max subtraction; scores are O(1))
  A^T  = V_aug-stationary matmul; appended ones column -> row 64 = denoms
  apn  = A^T rows * (1/denom) broadcast       (normalized, [f, t] layout)
  y^T += Wo[:, fsl]^T.T apn                   partial out-proj [1024, 2048]

Host sums the 4 partial y^T per batch, transposes, adds bo.

The emission order software-pipelines everything: QKV-projection and
out-projection matmul groups are interleaved into the exp-bound attention
stream as PE filler, so TensorE stays dense (HAM-warm) while ScalarE runs
back-to-back exp.

build_nc(iters=N) repeats the computation N times inside one NEFF so the
marginal per-iteration time can be measured through the large constant
axon dispatch overhead.
"""

import sys

sys.path.insert(0, "/opt/trn_rl_repo")

import numpy as np

import concourse.bass as bass
import concourse.mybir as mybir
import concourse.tile as tile
from concourse import bacc
from concourse.bass_utils import run_bass_kernel_spmd

F32 = mybir.dt.float32
F32R = mybir.dt.float32r
AF = mybir.ActivationFunctionType
ALU = mybir.AluOpType

B, S, D, H = 2, 2048, 1024, 16
MAX_SEQ_LEN = 5000
NCORES = 8
F = D // 4  # 256 features (4 heads) per core
HD = D // H  # 64
DT = D // 128  # 8 d-tiles
TT = S // 128  # 16 token tiles
QC = S // 512  # 4 q-chunks of 512
SCALE = 1.0 / np.sqrt(np.float32(D))  # 1/32


def build_nc(iters=1):
    nc = bacc.Bacc("TRN2", target_bir_lowering=False, debug=False, num_devices=NCORES)

    xt_d = nc.declare_dram_parameter("xt", [D, S], F32R, isOutput=False)
    wqt_d = nc.declare_dram_parameter("wqt", [D, F], F32R, isOutput=False)
    wkt_d = nc.declare_dram_parameter("wkt", [D, F], F32R, isOutput=False)
    wvt_d = nc.declare_dram_parameter("wvt", [D, F], F32R, isOutput=False)
    wot_d = nc.declare_dram_parameter("wot", [F, D], F32R, isOutput=False)
    pq_d = nc.declare_dram_parameter("pq", [F, S], F32, isOutput=False)
    pk_d = nc.declare_dram_parameter("pk", [F, S], F32, isOutput=False)
    pv_d = nc.declare_dram_parameter("pv", [S, F], F32, isOutput=False)
    yt_d = nc.declare_dram_parameter("yt", [D, S], F32, isOutput=True)

    with tile.TileContext(nc) as tc:
        with (
            tc.tile_pool(name="persist", bufs=1) as pp,
            tc.tile_pool(name="xin", bufs=1) as xp,
            tc.tile_pool(name="pe_in", bufs=2) as pep,
            tc.tile_pool(name="expp", bufs=3) as expp,
            tc.tile_pool(name="pvs", bufs=2) as pvsp,
            tc.tile_pool(name="rcs", bufs=2) as rcp,
            tc.tile_pool(name="stage", bufs=2) as stp,
            tc.tile_pool(name="yst", bufs=2) as ystp,
            tc.tile_pool(name="ps_sc", bufs=2, space="PSUM") as ps_sc,
            tc.tile_pool(name="ps_mm", bufs=4, space="PSUM") as ps_mm,
        ):
            env = {
                "pep": pep, "expp": expp, "pvsp": pvsp, "rcp": rcp,
                "stp": stp, "ystp": ystp, "ps_sc": ps_sc, "ps_mm": ps_mm,
                "xp": xp,
                "xt_d": xt_d, "pq_d": pq_d, "pk_d": pk_d, "pv_d": pv_d,
                "yt_d": yt_d,
            }
            env["wq"] = pp.tile([128, DT, F], F32R, name="wq")
            env["wk"] = pp.tile([128, DT, F], F32R, name="wk")
            env["wv"] = pp.tile([128, DT, F], F32R, name="wv")
            env["wo"] = pp.tile([128, 2, D], F32R, name="wo")
            env["qt"] = pp.tile([128, 2, S], F32R, name="qt")
            env["kt"] = pp.tile([128, 2, S], F32R, name="kt")
            env["vaug"] = pp.tile([128, TT, 4, HD + 1], F32R, name="vaug")
            env["apn"] = pp.tile([128, 2, S], F32R, name="apn")
            env["ones_bc"] = pp.tile([128, HD], F32R, name="ones_bc")
            ones_f32 = pp.tile([128, HD], F32)

            nc.vector.memset(ones_f32[:], 1.0)
            nc.vector.tensor_copy(env["ones_bc"][:], ones_f32[:])
            nc.vector.tensor_copy(
                env["vaug"][:, :, :, HD : HD + 1],
                ones_f32.rearrange("p (t h o) -> p t h o", t=TT, h=4),
            )
            nc.sync.dma_start(
                env["wv"][:], wvt_d.rearrange("(dt p) f -> p dt f", p=128)
            )
            nc.sync.dma_start(
                env["wq"][:], wqt_d.rearrange("(dt p) f -> p dt f", p=128)
            )
            nc.sync.dma_start(
                env["wk"][:], wkt_d.rearrange("(dt p) f -> p dt f", p=128)
            )
            nc.sync.dma_start(
                env["wo"][:], wot_d.rearrange("(ft p) m -> p ft m", p=128)
            )

            for _ in range(iters):
                body(nc, env)

    nc.compile()
    return nc


def body(nc, env):
    """Emit one software-pipelined forward pass."""
    pep, expp, pvsp, rcp = env["pep"], env["expp"], env["pvsp"], env["rcp"]
    stp, ystp, ps_sc, ps_mm = env["stp"], env["ystp"], env["ps_sc"], env["ps_mm"]
    wq, wk, wv, wo = env["wq"], env["wk"], env["wv"], env["wo"]
    qt, kt, vaug, apn = env["qt"], env["kt"], env["vaug"], env["apn"]
    ones_bc = env["ones_bc"]
    xt_d, pq_d, pk_d, pv_d, yt_d = (
        env["xt_d"], env["pq_d"], env["pk_d"], env["pv_d"], env["yt_d"],
    )

    xt = env["xp"].tile([128, DT, S], F32R, tag="xt")
    for dt in range(DT):
        nc.sync.dma_start(xt[:, dt], xt_d[dt * 128 : (dt + 1) * 128, :])

    # ---------- filler emitters (PE work woven into the attention stream) ----
    def v_chunk(tt):
        """V token-tile tt: accumulate 8 d-tiles, add host pe/bias term."""
        pvc = pep.tile([128, F], F32, tag="pvc")
        nc.sync.dma_start(pvc[:], pv_d[tt * 128 : (tt + 1) * 128, :])
        ps = ps_mm.tile([128, 512], F32, tag="mm")
        for dt in range(DT):
            nc.tensor.matmul(
                ps[:, :F],
                xt[:, dt, tt * 128 : (tt + 1) * 128],
                wv[:, dt],
                start=(dt == 0),
                stop=(dt == DT - 1),
            )
        nc.vector.tensor_tensor(
            vaug[:, tt, :, 0:HD],
            ps[:, :F].rearrange("p (h e) -> p h e", h=4),
            pvc.rearrange("p (h e) -> p h e", h=4),
            ALU.add,
        )

    def qkt_chunk(dst, w, pe_d, ft, qcc, tag):
        """One [128, 512] chunk of Q^T or K^T (f-tile ft, q-chunk qcc)."""
        pec = pep.tile([128, 512], F32, tag=tag)
        nc.sync.dma_start(
            pec[:], pe_d[ft * 128 : (ft + 1) * 128, qcc * 512 : (qcc + 1) * 512]
        )
        ps = ps_mm.tile([128, 512], F32, tag="mm")
        for dt in range(DT):
            nc.tensor.matmul(
                ps[:],
                w[:, dt, ft * 128 : (ft + 1) * 128],
                xt[:, dt, qcc * 512 : (qcc + 1) * 512],
                start=(dt == 0),
                stop=(dt == DT - 1),
            )
        nc.vector.tensor_tensor(
            dst[:, ft, qcc * 512 : (qcc + 1) * 512], ps[:], pec[:], ALU.add
        )

    def outproj_chunk(mt, tcc):
        ps = ps_mm.tile([128, 512], F32, tag="mm")
        for ft in range(2):
            nc.tensor.matmul(
                ps[:],
                wo[:, ft, mt * 128 : (mt + 1) * 128],
                apn[:, ft, tcc * 512 : (tcc + 1) * 512],
                start=(ft == 0),
                stop=(ft == 1),
            )
        yst = ystp.tile([128, 512], F32, tag="yst")
        nc.scalar.activation(yst[:], ps[:], AF.Copy, bias=0.0)
        nc.sync.dma_start(
            yt_d[mt * 128 : (mt + 1) * 128, tcc * 512 : (tcc + 1) * 512], yst[:]
        )

    # ---------- attention steps ----------
    def attn_step(p, qc, k, pv0, pv1):
        sc = ps_sc.tile([128, 1024], F32, tag="sc")
        nc.tensor.matmul(
            sc[:, 0:512],
            kt[0:64, p, k * 128 : (k + 1) * 128],
            qt[0:64, p, qc * 512 : (qc + 1) * 512],
            start=True,
            stop=True,
        )
        nc.tensor.matmul(
            sc[:, 512:1024],
            kt[64:128, p, k * 128 : (k + 1) * 128],
            qt[64:128, p, qc * 512 : (qc + 1) * 512],
            start=True,
            stop=True,
        )
        ex = expp.tile([128, 1024], F32R, tag="ex")
        nc.scalar.activation(ex[:], sc[:], AF.Exp, scale=float(SCALE))
        nc.tensor.matmul(
            pv0[0:65, :],
            vaug[:, k, 2 * p, :],
            ex[:, 0:512],
            start=(k == 0),
            stop=(k == TT - 1),
        )
        nc.tensor.matmul(
            pv1[0:65, :],
            vaug[:, k, 2 * p + 1, :],
            ex[:, 512:1024],
            start=(k == 0),
            stop=(k == TT - 1),
        )

    def normalize(p, qc, h, pv):
        pvs = pvsp.tile([65, 512], F32, tag="pvs")
        nc.vector.tensor_copy(pvs[:], pv[0:65, :])
        rc = rcp.tile([65, 512], F32R, tag="rc")
        with nc.allow_low_precision(reason="f32r for matmul rhs"):
            nc.vector.reciprocal(rc[64:65, :], pvs[64:65, :])
        bc = ps_sc.tile([128, 1024], F32, tag="sc")
        nc.tensor.matmul(
            bc[0:64, 0:512], ones_bc[64:65, :], rc[64:65, :], start=True, stop=True
        )
        if h % 2 == 0:
            nc.vector.tensor_tensor(
                apn[0:64, p, qc * 512 : (qc + 1) * 512],
                pvs[0:64, :],
                bc[0:64, 0:512],
                ALU.mult,
            )
        else:
            st = stp.tile([64, 512], F32R, tag="st")
            nc.vector.tensor_tensor(st[:], pvs[0:64, :], bc[0:64, 0:512], ALU.mult)
            nc.sync.dma_start(apn[64:128, p, qc * 512 : (qc + 1) * 512], st[:])

    # ---------- filler schedule ----------
    # pair 0 needs, before global step s = qc*TT + k:
    #   V(tt=k) for attnV, KT0 chunk k//4 for scores, QT0 chunk qc.
    # Prologue: V(0), KT0c0, QT0c0.  Due-driven emission handles the rest;
    # extra (non-due) filler is paced evenly across the remaining steps.
    v_chunk(0)
    qkt_chunk(kt, wk, pk_d, 0, 0, "pkc")
    qkt_chunk(qt, wq, pq_d, 0, 0, "pqc")

    due = []  # (due_step, emit_fn) for pair 0; due_step = qc*TT + k
    for ttc in range(1, TT):
        due.append((ttc, lambda ttc=ttc: v_chunk(ttc)))
    for j in range(1, QC):
        due.append((4 * j, lambda j=j: qkt_chunk(kt, wk, pk_d, 0, j, "pkc")))
        due.append(((j - 1) * TT + 10, lambda j=j: qkt_chunk(qt, wq, pq_d, 0, j, "pqc")))
    pace = [  # no hard deadline within pair 0: ft=1 projections
        (lambda j=j: qkt_chunk(kt, wk, pk_d, 1, j, "pkc")) for j in range(QC)
    ] + [
        (lambda j=j: qkt_chunk(qt, wq, pq_d, 1, j, "pqc")) for j in range(QC)
    ]
    due.sort(key=lambda t: t[0])
    due_i = 0
    pace_i = 0
    for qc in range(QC):
        pv0 = ps_mm.tile([128, 512], F32, tag="mm")
        pv1 = ps_mm.tile([128, 512], F32, tag="mm")
        for k in range(TT):
            s = qc * TT + k
            while due_i < len(due) and due[due_i][0] <= s:
                due[due_i][1]()
                due_i += 1
            if s % 7 == 6 and pace_i < len(pace):
                pace[pace_i]()
                pace_i += 1
            attn_step(0, qc, k, pv0, pv1)
        normalize(0, qc, 0, pv0)
        normalize(0, qc, 1, pv1)
    while pace_i < len(pace):
        pace[pace_i]()
        pace_i += 1

    # pair 1: interleave out-projection chunks for finished t-chunks.
    # outproj(:, tcc) needs apn ft0 (done) and ft1 at tcc -> after qc=tcc.
    op_queue = []
    for qc in range(QC):
        pv0 = ps_mm.tile([128, 512], F32, tag="mm")
        pv1 = ps_mm.tile([128, 512], F32, tag="mm")
        for k in range(TT):
            if k % 2 == 1 and op_queue:
                op_queue.pop(0)()
            attn_step(1, qc, k, pv0, pv1)
        normalize(1, qc, 0, pv0)
        normalize(1, qc, 1, pv1)
        op_queue.extend(
            [lambda mt=mt, qc=qc: outproj_chunk(mt, qc) for mt in range(DT)]
        )
    for f in op_queue:
        f()


_NC = {}


def _get_nc(iters=1):
    if iters not in _NC:
        _NC[iters] = build_nc(iters)
    return _NC[iters]


def _sinusoid_pe():
    pos = np.arange(MAX_SEQ_LEN, dtype=np.float32)[:, None]
    div = np.exp(
        np.arange(0, D, 2, dtype=np.float32) * np.float32(-np.log(10000.0) / D)
    )
    ang = pos * div[None, :]
    pe = np.stack([np.sin(ang), np.cos(ang)], axis=-1).reshape(MAX_SEQ_LEN, D)
    return pe.astype(np.float32)


def make_in_maps(x, rel_emb, alpha, Wq, bq, Wk, bk, Wv, bv, Wo, bo):
    alpha = np.float32(alpha)
    abs_pe = _sinusoid_pe()[:S]
    rel_pe = rel_emb[MAX_SEQ_LEN - S : MAX_SEQ_LEN]
    pe = (alpha * abs_pe + (np.float32(1.0) - alpha) * rel_pe).astype(np.float32)

    in_maps = []
    for c in range(NCORES):
        b, g = divmod(c, 4)
        fsl = slice(g * F, (g + 1) * F)
        wq_s, wk_s, wv_s = Wq[fsl], Wk[fsl], Wv[fsl]
        in_maps.append(
            {
                "xt": np.ascontiguousarray(x[b].T),
                "wqt": np.ascontiguousarray(wq_s.T),
                "wkt": np.ascontiguousarray(wk_s.T),
                "wvt": np.ascontiguousarray(wv_s.T),
                "wot": np.ascontiguousarray(Wo[:, fsl].T),
                "pq": np.ascontiguousarray((pe @ wq_s.T + bq[fsl]).T),
                "pk": np.ascontiguousarray((pe @ wk_s.T + bk[fsl]).T),
                "pv": np.ascontiguousarray(pe @ wv_s.T + bv[fsl]),
            }
        )
    return in_maps


def unshard(results, bo):
    y = np.empty((B, S, D), dtype=np.float32)
    for b in range(B):
        acc = results[4 * b]["yt"].astype(np.float32).copy()
        for g in range(1, 4):
            acc += results[4 * b + g]["yt"]
        y[b] = acc.T + bo
    return y


def kernel(x, rel_emb, alpha, Wq, bq, Wk, bk, Wv, bv, Wo, bo, **kw):
    x = np.asarray(x, dtype=np.float32)
    args = [
        np.asarray(a, dtype=np.float32)
        for a in (rel_emb, alpha, Wq, bq, Wk, bk, Wv, bv, Wo, bo)
    ]
    nc = _get_nc()
    in_maps = make_in_maps(x, *args)
    res = run_bass_kernel_spmd(nc, in_maps, core_ids=list(range(NCORES)))
    return unshard(res.results, args[-1])


# revision 9
# speedup vs baseline: 6.5230x; 6.5230x over previous
"""Trainium2 Bass kernel for nn_MixtureAttention (B=2, S=2048, D=1024, H=16).

Sharding: 8 cores = 2 batches x 4 head-groups (4 heads each, Megatron-style
tensor parallel). Each core computes, for its batch b and feature slice
fsl (256 features = 4 heads):

  Q^T  = (Wq[fsl] . x^T) + (pe Wq^T + bq)^T   [256, 2048]  (float32r matmuls)
  K^T  likewise; V = x Wv^T + (pe Wv^T + bv)  [2048, 256] natural layout
  S^T  = per-head scores in [k_tok, q] PSUM tiles (head pair row-packed)
  P^T  = exp(S^T / 32)            (no max subtraction; scores are O(1))
  A^T  = V_aug-stationary matmul; appended ones column -> row 64 = denoms
  apn  = A^T rows * (1/denom) broadcast       (normalized, [f, t] layout)
  y^T += Wo[:, fsl]^T.T apn                   partial out-proj [1024, 2048]

Host sums the 4 partial y^T per batch, transposes, adds bo.

The emission order software-pipelines everything: QKV-projection and
out-projection matmul groups are interleaved into the exp-bound attention
stream as PE filler, so TensorE stays dense (HAM-warm) while ScalarE runs
back-to-back exp.

build_nc(iters=N) repeats the computation N times inside one NEFF so the
marginal per-iteration time can be measured through the large constant
axon dispatch overhead.
"""

import sys

sys.path.insert(0, "/opt/trn_rl_repo")

import numpy as np

import concourse.bass as bass
import concourse.mybir as mybir
import concourse.tile as tile
from concourse import bacc
from concourse.bass_utils import run_bass_kernel_spmd

F32 = mybir.dt.float32
F32R = mybir.dt.float32r
AF = mybir.ActivationFunctionType
ALU = mybir.AluOpType

B, S, D, H = 2, 2048, 1024, 16
MAX_SEQ_LEN = 5000
NCORES = 8
F = D // 4  # 256 features (4 heads) per core
HD = D // H  # 64
DT = D // 128  # 8 d-tiles
TT = S // 128  # 16 token tiles
QC = S // 512  # 4 q-chunks of 512
SCALE = 1.0 / np.sqrt(np.float32(D))  # 1/32


def build_nc(iters=1):
    nc = bacc.Bacc("TRN2", target_bir_lowering=False, debug=False, num_devices=NCORES)

    xt_d = nc.declare_dram_parameter("xt", [D, S], F32R, isOutput=False)
    wqt_d = nc.declare_dram_parameter("wqt", [D, F], F32R, isOutput=False)
    wkt_d = nc.declare_dram_parameter("wkt", [D, F], F32R, isOutput=False)
    wvt_d = nc.declare_dram_parameter("wvt", [D, F], F32R, isOutput=False)
    wot_d = nc.declare_dram_parameter("wot", [F, D], F32R, isOutput=False)
    pq_d = nc.declare_dram_parameter("pq", [F, S], F32, isOutput=False)
    pk_d = nc.declare_dram_parameter("pk", [F, S], F32, isOutput=False)
    pv_d = nc.declare_dram_parameter("pv", [S, F], F32, isOutput=False)
    yt_d = nc.declare_dram_parameter("yt", [D, S], F32, isOutput=True)

    with tile.TileContext(nc) as tc:
        with (
            tc.tile_pool(name="persist", bufs=1) as pp,
            tc.tile_pool(name="xin", bufs=1) as xp,
            tc.tile_pool(name="pe_in", bufs=2) as pep,
            tc.tile_pool(name="expp", bufs=3) as expp,
            tc.tile_pool(name="pvs", bufs=2) as pvsp,
            tc.tile_pool(name="rcs", bufs=2) as rcp,
            tc.tile_pool(name="stage", bufs=2) as stp,
            tc.tile_pool(name="yst", bufs=2) as ystp,
            tc.tile_pool(name="ps_sc", bufs=2, space="PSUM") as ps_sc,
            tc.tile_pool(name="ps_mm", bufs=4, space="PSUM") as ps_mm,
        ):
            env = {
                "pep": pep, "expp": expp, "pvsp": pvsp, "rcp": rcp,
                "stp": stp, "ystp": ystp, "ps_sc": ps_sc, "ps_mm": ps_mm,
                "xp": xp,
                "xt_d": xt_d, "pq_d": pq_d, "pk_d": pk_d, "pv_d": pv_d,
                "yt_d": yt_d,
            }
            env["wq"] = pp.tile([128, DT, F], F32R, name="wq")
            env["wk"] = pp.tile([128, DT, F], F32R, name="wk")
            env["wv"] = pp.tile([128, DT, F], F32R, name="wv")
            env["wo"] = pp.tile([128, 2, D], F32R, name="wo")
            env["qt"] = pp.tile([128, 2, S], F32R, name="qt")
            env["kt"] = pp.tile([128, 2, S], F32R, name="kt")
            env["vaug"] = pp.tile([128, TT, 4, HD + 1], F32R, name="vaug")
            env["apn"] = pp.tile([128, 2, S], F32R, name="apn")
            env["ones_bc"] = pp.tile([128, HD], F32R, name="ones_bc")
            ones_f32 = pp.tile([128, HD], F32)

            nc.vector.memset(ones_f32[:], 1.0)
            nc.vector.tensor_copy(env["ones_bc"][:], ones_f32[:])
            nc.vector.tensor_copy(
                env["vaug"][:, :, :, HD : HD + 1],
                ones_f32.rearrange("p (t h o) -> p t h o", t=TT, h=4),
            )
            nc.sync.dma_start(
                env["wv"][:], wvt_d.rearrange("(dt p) f -> p dt f", p=128)
            )
            nc.sync.dma_start(
                env["wq"][:], wqt_d.rearrange("(dt p) f -> p dt f", p=128)
            )
            nc.sync.dma_start(
                env["wk"][:], wkt_d.rearrange("(dt p) f -> p dt f", p=128)
            )
            nc.sync.dma_start(
                env["wo"][:], wot_d.rearrange("(ft p) m -> p ft m", p=128)
            )

            for _ in range(iters):
                body(nc, env)

    nc.compile()
    return nc


def body(nc, env):
    """Emit one software-pipelined forward pass."""
    pep, expp, pvsp, rcp = env["pep"], env["expp"], env["pvsp"], env["rcp"]
    stp, ystp, ps_sc, ps_mm = env["stp"], env["ystp"], env["ps_sc"], env["ps_mm"]
    wq, wk, wv, wo = env["wq"], env["wk"], env["wv"], env["wo"]
    qt, kt, vaug, apn = env["qt"], env["kt"], env["vaug"], env["apn"]
    ones_bc = env["ones_bc"]
    xt_d, pq_d, pk_d, pv_d, yt_d = (
        env["xt_d"], env["pq_d"], env["pk_d"], env["pv_d"], env["yt_d"],
    )

    xt = env["xp"].tile([128, DT, S], F32R, tag="xt")
    for dt in range(DT):
        nc.sync.dma_start(xt[:, dt], xt_d[dt * 128 : (dt + 1) * 128, :])

    # ---------- filler emitters (PE work woven into the attention stream) ----
    def v_chunk(tt):
        """V token-tile tt: accumulate 8 d-tiles, add host pe/bias term."""
        pvc = pep.tile([128, F], F32, tag="pvc")
        nc.sync.dma_start(pvc[:], pv_d[tt * 128 : (tt + 1) * 128, :])
        ps = ps_mm.tile([128, 512], F32, tag="mm")
        for dt in range(DT):
            nc.tensor.matmul(
                ps[:, :F],
                xt[:, dt, tt * 128 : (tt + 1) * 128],
                wv[:, dt],
                start=(dt == 0),
                stop=(dt == DT - 1),
            )
        nc.vector.tensor_tensor(
            vaug[:, tt, :, 0:HD],
            ps[:, :F].rearrange("p (h e) -> p h e", h=4),
            pvc.rearrange("p (h e) -> p h e", h=4),
            ALU.add,
        )

    def qkt_chunk(dst, w, pe_d, ft, qcc, tag):
        """One [128, 512] chunk of Q^T or K^T (f-tile ft, q-chunk qcc)."""
        pec = pep.tile([128, 512], F32, tag=tag)
        nc.sync.dma_start(
            pec[:], pe_d[ft * 128 : (ft + 1) * 128, qcc * 512 : (qcc + 1) * 512]
        )
        ps = ps_mm.tile([128, 512], F32, tag="mm")
        for dt in range(DT):
            nc.tensor.matmul(
                ps[:],
                w[:, dt, ft * 128 : (ft + 1) * 128],
                xt[:, dt, qcc * 512 : (qcc + 1) * 512],
                start=(dt == 0),
                stop=(dt == DT - 1),
            )
        nc.vector.tensor_tensor(
            dst[:, ft, qcc * 512 : (qcc + 1) * 512], ps[:], pec[:], ALU.add
        )

    def outproj_chunk(mt, tcc):
        ps = ps_mm.tile([128, 512], F32, tag="mm")
        for ft in range(2):
            nc.tensor.matmul(
                ps[:],
                wo[:, ft, mt * 128 : (mt + 1) * 128],
                apn[:, ft, tcc * 512 : (tcc + 1) * 512],
                start=(ft == 0),
                stop=(ft == 1),
            )
        yst = ystp.tile([128, 512], F32, tag="yst")
        nc.vector.tensor_copy(yst[:], ps[:])
        nc.sync.dma_start(
            yt_d[mt * 128 : (mt + 1) * 128, tcc * 512 : (tcc + 1) * 512], yst[:]
        )

    # ---------- attention steps ----------
    def attn_step(p, qc, k, pv0, pv1):
        sc = ps_sc.tile([128, 1024], F32, tag="sc")
        nc.tensor.matmul(
            sc[:, 0:512],
            kt[0:64, p, k * 128 : (k + 1) * 128],
            qt[0:64, p, qc * 512 : (qc + 1) * 512],
            start=True,
            stop=True,
        )
        nc.tensor.matmul(
            sc[:, 512:1024],
            kt[64:128, p, k * 128 : (k + 1) * 128],
            qt[64:128, p, qc * 512 : (qc + 1) * 512],
            start=True,
            stop=True,
        )
        ex = expp.tile([128, 1024], F32R, tag="ex")
        nc.scalar.activation(ex[:], sc[:], AF.Exp, scale=float(SCALE))
        nc.tensor.matmul(
            pv0[0:65, :],
            vaug[:, k, 2 * p, :],
            ex[:, 0:512],
            start=(k == 0),
            stop=(k == TT - 1),
        )
        nc.tensor.matmul(
            pv1[0:65, :],
            vaug[:, k, 2 * p + 1, :],
            ex[:, 512:1024],
            start=(k == 0),
            stop=(k == TT - 1),
        )

    def normalize_a(pv):
        """Prompt DVE part: copy PSUM accumulator out, reciprocal of denoms."""
        pvs = pvsp.tile([65, 512], F32, tag="pvs")
        nc.vector.tensor_copy(pvs[:], pv[0:65, :])
        rc = rcp.tile([65, 512], F32R, tag="rc")
        with nc.allow_low_precision(reason="f32r for matmul rhs"):
            nc.vector.reciprocal(rc[64:65, :], pvs[64:65, :])
        return pvs, rc

    def normalize_b(p, qc, h, pvs, rc):
        """Deferred PE+DVE part: broadcast recip, scale, store to apn."""
        bc = ps_sc.tile([128, 1024], F32, tag="sc")
        nc.tensor.matmul(
            bc[0:64, 0:512], ones_bc[64:65, :], rc[64:65, :], start=True, stop=True
        )
        if h % 2 == 0:
            nc.vector.tensor_tensor(
                apn[0:64, p, qc * 512 : (qc + 1) * 512],
                pvs[0:64, :],
                bc[0:64, 0:512],
                ALU.mult,
            )
        else:
            st = stp.tile([64, 512], F32R, tag="st")
            nc.vector.tensor_tensor(st[:], pvs[0:64, :], bc[0:64, 0:512], ALU.mult)
            nc.sync.dma_start(apn[64:128, p, qc * 512 : (qc + 1) * 512], st[:])

    # ---------- filler schedule ----------
    # pair 0 needs, before global step s = qc*TT + k:
    #   V(tt=k) for attnV, KT0 chunk k//4 for scores, QT0 chunk qc.
    # Prologue: V(0), KT0c0, QT0c0.  Due-driven emission handles the rest;
    # extra (non-due) filler is paced evenly across the remaining steps.
    v_chunk(0)
    qkt_chunk(kt, wk, pk_d, 0, 0, "pkc")
    qkt_chunk(qt, wq, pq_d, 0, 0, "pqc")

    due = []  # (due_step, emit_fn) for pair 0; due_step = qc*TT + k
    for ttc in range(1, TT):
        due.append((ttc, lambda ttc=ttc: v_chunk(ttc)))
    for j in range(1, QC):
        due.append((4 * j, lambda j=j: qkt_chunk(kt, wk, pk_d, 0, j, "pkc")))
        due.append(((j - 1) * TT + 10, lambda j=j: qkt_chunk(qt, wq, pq_d, 0, j, "pqc")))
    pace = [  # no hard deadline within pair 0: ft=1 projections
        (lambda j=j: qkt_chunk(kt, wk, pk_d, 1, j, "pkc")) for j in range(QC)
    ] + [
        (lambda j=j: qkt_chunk(qt, wq, pq_d, 1, j, "pqc")) for j in range(QC)
    ]
    due.sort(key=lambda t: t[0])
    due_i = 0
    pace_i = 0
    norm_q = []
    for qc in range(QC):
        pv0 = ps_mm.tile([128, 512], F32, tag="mm")
        pv1 = ps_mm.tile([128, 512], F32, tag="mm")
        for k in range(TT):
            s = qc * TT + k
            while due_i < len(due) and due[due_i][0] <= s:
                due[due_i][1]()
                due_i += 1
            if k in (2, 4) and norm_q:
                norm_q.pop(0)()
            if s % 7 == 6 and pace_i < len(pace):
                pace[pace_i]()
                pace_i += 1
            attn_step(0, qc, k, pv0, pv1)
        a0 = normalize_a(pv0)
        a1 = normalize_a(pv1)
        norm_q.append(lambda qc=qc, a=a0: normalize_b(0, qc, 0, *a))
        norm_q.append(lambda qc=qc, a=a1: normalize_b(0, qc, 1, *a))
    while pace_i < len(pace):
        pace[pace_i]()
        pace_i += 1
    for f in norm_q:
        f()
    norm_q = []

    # pair 1: interleave out-projection chunks for finished t-chunks.
    # outproj(:, tcc) needs apn ft0 (done) and ft1 at tcc -> after qc=tcc.
    op_queue = []
    for qc in range(QC):
        pv0 = ps_mm.tile([128, 512], F32, tag="mm")
        pv1 = ps_mm.tile([128, 512], F32, tag="mm")
        for k in range(TT):
            if k in (2, 4) and norm_q:
                norm_q.pop(0)()
            if k >= 5 and k % 2 == 1 and op_queue:
                op_queue.pop(0)()
            attn_step(1, qc, k, pv0, pv1)
        a0 = normalize_a(pv0)
        a1 = normalize_a(pv1)
        norm_q.append(lambda qc=qc, a=a0: normalize_b(1, qc, 0, *a))
        norm_q.append(lambda qc=qc, a=a1: normalize_b(1, qc, 1, *a))
        if qc > 0:
            op_queue.extend(
                [lambda mt=mt, qc=qc: outproj_chunk(mt, qc - 1) for mt in range(DT)]
            )
    for f in norm_q:
        f()
    for f in op_queue:
        f()
    for mt in range(DT):
        outproj_chunk(mt, QC - 1)


_NC = {}


def _get_nc(iters=1):
    if iters not in _NC:
        _NC[iters] = build_nc(iters)
    return _NC[iters]


def _sinusoid_pe():
    pos = np.arange(MAX_SEQ_LEN, dtype=np.float32)[:, None]
    div = np.exp(
        np.arange(0, D, 2, dtype=np.float32) * np.float32(-np.log(10000.0) / D)
    )
    ang = pos * div[None, :]
    pe = np.stack([np.sin(ang), np.cos(ang)], axis=-1).reshape(MAX_SEQ_LEN, D)
    return pe.astype(np.float32)


def make_in_maps(x, rel_emb, alpha, Wq, bq, Wk, bk, Wv, bv, Wo, bo):
    alpha = np.float32(alpha)
    abs_pe = _sinusoid_pe()[:S]
    rel_pe = rel_emb[MAX_SEQ_LEN - S : MAX_SEQ_LEN]
    pe = (alpha * abs_pe + (np.float32(1.0) - alpha) * rel_pe).astype(np.float32)

    in_maps = []
    for c in range(NCORES):
        b, g = divmod(c, 4)
        fsl = slice(g * F, (g + 1) * F)
        wq_s, wk_s, wv_s = Wq[fsl], Wk[fsl], Wv[fsl]
        in_maps.append(
            {
                "xt": np.ascontiguousarray(x[b].T),
                "wqt": np.ascontiguousarray(wq_s.T),
                "wkt": np.ascontiguousarray(wk_s.T),
                "wvt": np.ascontiguousarray(wv_s.T),
                "wot": np.ascontiguousarray(Wo[:, fsl].T),
                "pq": np.ascontiguousarray((pe @ wq_s.T + bq[fsl]).T),
                "pk": np.ascontiguousarray((pe @ wk_s.T + bk[fsl]).T),
                "pv": np.ascontiguousarray(pe @ wv_s.T + bv[fsl]),
            }
        )
    return in_maps


def unshard(results, bo):
    y = np.empty((B, S, D), dtype=np.float32)
    for b in range(B):
        acc = results[4 * b]["yt"].astype(np.float32).copy()
        for g in range(1, 4):
            acc += results[4 * b + g]["yt"]
        y[b] = acc.T + bo
    return y


def kernel(x, rel_emb, alpha, Wq, bq, Wk, bk, Wv, bv, Wo, bo, **kw):
    x = np.asarray(x, dtype=np.float32)
    args = [
        np.asarray(a, dtype=np.float32)
        for a in (rel_emb, alpha, Wq, bq, Wk, bk, Wv, bv, Wo, bo)
    ]
    nc = _get_nc()
    in_maps = make_in_maps(x, *args)
    res = run_bass_kernel_spmd(nc, in_maps, core_ids=list(range(NCORES)))
    return unshard(res.results, args[-1])


# revision 11
# speedup vs baseline: 8.1872x; 1.2551x over previous
"""Trainium2 Bass kernel for nn_MixtureAttention (B=2, S=2048, D=1024, H=16).

Sharding: 8 cores = 2 batches x 4 head-groups (4 heads each, Megatron-style
tensor parallel). Each core computes, for its batch b and feature slice
fsl (256 features = 4 heads):

  Q^T  = (Wq[fsl] . x^T) + (pe Wq^T + bq)^T   [256, 2048]  (float32r matmuls)
  K^T  likewise; V = x Wv^T + (pe Wv^T + bv)  [2048, 256] natural layout
  S^T  = per-head scores in [k_tok, q] PSUM tiles (head pair row-packed)
  P^T  = exp(S^T / 32)            (no max subtraction; scores are O(1))
  A^T  = V_aug-stationary matmul; appended ones column -> row 64 = denoms
  apn  = A^T rows * (1/denom) broadcast       (normalized, [f, t] layout)
  y^T += Wo[:, fsl]^T.T apn                   partial out-proj [1024, 2048]

Host sums the 4 partial y^T per batch, transposes, adds bo.

The emission order software-pipelines everything: QKV-projection and
out-projection matmul groups are interleaved into the exp-bound attention
stream as PE filler, so TensorE stays dense (HAM-warm) while ScalarE runs
back-to-back exp.

build_nc(iters=N) repeats the computation N times inside one NEFF so the
marginal per-iteration time can be measured through the large constant
axon dispatch overhead.
"""

import sys

sys.path.insert(0, "/opt/trn_rl_repo")

import ml_dtypes
import numpy as np

import concourse.bass as bass
import concourse.mybir as mybir
import concourse.tile as tile
from concourse import bacc
from concourse.bass_utils import run_bass_kernel_spmd

F32 = mybir.dt.float32
F32R = mybir.dt.float32r
BF16 = mybir.dt.bfloat16
AF = mybir.ActivationFunctionType
ALU = mybir.AluOpType

B, S, D, H = 2, 2048, 1024, 16
MAX_SEQ_LEN = 5000
NCORES = 8
F = D // 4  # 256 features (4 heads) per core
HD = D // H  # 64
DT = D // 128  # 8 d-tiles
TT = S // 128  # 16 token tiles
QC = S // 512  # 4 q-chunks of 512
SCALE = 1.0 / np.sqrt(np.float32(D))  # 1/32


def build_nc(iters=1):
    nc = bacc.Bacc("TRN2", target_bir_lowering=False, debug=False, num_devices=NCORES)

    xt_d = nc.declare_dram_parameter("xt", [D, S], BF16, isOutput=False)
    wqt_d = nc.declare_dram_parameter("wqt", [D, F], BF16, isOutput=False)
    wkt_d = nc.declare_dram_parameter("wkt", [D, F], BF16, isOutput=False)
    wvt_d = nc.declare_dram_parameter("wvt", [D, F], BF16, isOutput=False)
    wot_d = nc.declare_dram_parameter("wot", [F, D], F32R, isOutput=False)
    pq_d = nc.declare_dram_parameter("pq", [F, S], F32, isOutput=False)
    pk_d = nc.declare_dram_parameter("pk", [F, S], F32, isOutput=False)
    pv_d = nc.declare_dram_parameter("pv", [S, F], F32, isOutput=False)
    yt_d = nc.declare_dram_parameter("yt", [D, S], F32, isOutput=True)

    with tile.TileContext(nc) as tc:
        with (
            tc.tile_pool(name="persist", bufs=1) as pp,
            tc.tile_pool(name="xin", bufs=1) as xp,
            tc.tile_pool(name="pe_in", bufs=2) as pep,
            tc.tile_pool(name="expp", bufs=3) as expp,
            tc.tile_pool(name="pvs", bufs=2) as pvsp,
            tc.tile_pool(name="rcs", bufs=2) as rcp,
            tc.tile_pool(name="stage", bufs=2) as stp,
            tc.tile_pool(name="yst", bufs=2) as ystp,
            tc.tile_pool(name="ps_sc", bufs=2, space="PSUM") as ps_sc,
            tc.tile_pool(name="ps_mm", bufs=4, space="PSUM") as ps_mm,
        ):
            env = {
                "pep": pep, "expp": expp, "pvsp": pvsp, "rcp": rcp,
                "stp": stp, "ystp": ystp, "ps_sc": ps_sc, "ps_mm": ps_mm,
                "xp": xp,
                "xt_d": xt_d, "pq_d": pq_d, "pk_d": pk_d, "pv_d": pv_d,
                "yt_d": yt_d,
            }
            env["wq"] = pp.tile([128, DT, F], BF16, name="wq")
            env["wk"] = pp.tile([128, DT, F], BF16, name="wk")
            env["wv"] = pp.tile([128, DT, F], BF16, name="wv")
            env["wo"] = pp.tile([128, 2, D], F32R, name="wo")
            env["qt"] = pp.tile([128, 2, S], BF16, name="qt")
            env["kt"] = pp.tile([128, 2, S], BF16, name="kt")
            env["vaug"] = pp.tile([128, TT, 4, HD + 1], BF16, name="vaug")
            env["apn"] = pp.tile([128, 2, S], F32R, name="apn")
            env["ones_bc"] = pp.tile([128, HD], F32R, name="ones_bc")
            ones_f32 = pp.tile([128, HD], F32)

            nc.vector.memset(ones_f32[:], 1.0)
            nc.vector.tensor_copy(env["ones_bc"][:], ones_f32[:])
            nc.vector.tensor_copy(
                env["vaug"][:, :, :, HD : HD + 1],
                ones_f32.rearrange("p (t h o) -> p t h o", t=TT, h=4),
            )
            nc.sync.dma_start(
                env["wv"][:], wvt_d.rearrange("(dt p) f -> p dt f", p=128)
            )
            nc.sync.dma_start(
                env["wq"][:], wqt_d.rearrange("(dt p) f -> p dt f", p=128)
            )
            nc.sync.dma_start(
                env["wk"][:], wkt_d.rearrange("(dt p) f -> p dt f", p=128)
            )
            nc.sync.dma_start(
                env["wo"][:], wot_d.rearrange("(ft p) m -> p ft m", p=128)
            )

            for _ in range(iters):
                body(nc, env)

    nc.compile()
    return nc


def body(nc, env):
    """Emit one software-pipelined forward pass."""
    pep, expp, pvsp, rcp = env["pep"], env["expp"], env["pvsp"], env["rcp"]
    stp, ystp, ps_sc, ps_mm = env["stp"], env["ystp"], env["ps_sc"], env["ps_mm"]
    wq, wk, wv, wo = env["wq"], env["wk"], env["wv"], env["wo"]
    qt, kt, vaug, apn = env["qt"], env["kt"], env["vaug"], env["apn"]
    ones_bc = env["ones_bc"]
    xt_d, pq_d, pk_d, pv_d, yt_d = (
        env["xt_d"], env["pq_d"], env["pk_d"], env["pv_d"], env["yt_d"],
    )

    xt = env["xp"].tile([128, DT, S], BF16, tag="xt")
    for dt in range(DT):
        nc.sync.dma_start(xt[:, dt], xt_d[dt * 128 : (dt + 1) * 128, :])

    # ---------- filler emitters (PE work woven into the attention stream) ----
    def v_chunk(tt):
        """V token-tile tt: accumulate 8 d-tiles, add host pe/bias term."""
        pvc = pep.tile([128, F], F32, tag="pvc")
        nc.sync.dma_start(pvc[:], pv_d[tt * 128 : (tt + 1) * 128, :])
        ps = ps_mm.tile([128, 512], F32, tag="mm")
        for dt in range(DT):
            nc.tensor.matmul(
                ps[:, :F],
                xt[:, dt, tt * 128 : (tt + 1) * 128],
                wv[:, dt],
                start=(dt == 0),
                stop=(dt == DT - 1),
            )
        nc.vector.tensor_tensor(
            vaug[:, tt, :, 0:HD],
            ps[:, :F].rearrange("p (h e) -> p h e", h=4),
            pvc.rearrange("p (h e) -> p h e", h=4),
            ALU.add,
        )

    def qkt_chunk(dst, w, pe_d, ft, qcc, tag):
        """One [128, 512] chunk of Q^T or K^T (f-tile ft, q-chunk qcc)."""
        pec = pep.tile([128, 512], F32, tag=tag)
        nc.sync.dma_start(
            pec[:], pe_d[ft * 128 : (ft + 1) * 128, qcc * 512 : (qcc + 1) * 512]
        )
        ps = ps_mm.tile([128, 512], F32, tag="mm")
        for dt in range(DT):
            nc.tensor.matmul(
                ps[:],
                w[:, dt, ft * 128 : (ft + 1) * 128],
                xt[:, dt, qcc * 512 : (qcc + 1) * 512],
                start=(dt == 0),
                stop=(dt == DT - 1),
            )
        nc.vector.tensor_tensor(
            dst[:, ft, qcc * 512 : (qcc + 1) * 512], ps[:], pec[:], ALU.add
        )

    def outproj_chunk(mt, tcc):
        ps = ps_mm.tile([128, 512], F32, tag="mm")
        for ft in range(2):
            nc.tensor.matmul(
                ps[:],
                wo[:, ft, mt * 128 : (mt + 1) * 128],
                apn[:, ft, tcc * 512 : (tcc + 1) * 512],
                start=(ft == 0),
                stop=(ft == 1),
            )
        yst = ystp.tile([128, 512], F32, tag="yst")
        nc.vector.tensor_copy(yst[:], ps[:])
        nc.sync.dma_start(
            yt_d[mt * 128 : (mt + 1) * 128, tcc * 512 : (tcc + 1) * 512], yst[:]
        )

    # ---------- attention steps ----------
    def attn_step(p, qc, k, pv0, pv1):
        sc = ps_sc.tile([128, 1024], F32, tag="sc")
        nc.tensor.matmul(
            sc[:, 0:512],
            kt[0:64, p, k * 128 : (k + 1) * 128],
            qt[0:64, p, qc * 512 : (qc + 1) * 512],
            start=True,
            stop=True,
        )
        nc.tensor.matmul(
            sc[:, 512:1024],
            kt[64:128, p, k * 128 : (k + 1) * 128],
            qt[64:128, p, qc * 512 : (qc + 1) * 512],
            start=True,
            stop=True,
        )
        ex = expp.tile([128, 1024], BF16, tag="ex")
        nc.scalar.activation(ex[:], sc[:], AF.Exp, scale=float(SCALE))
        nc.tensor.matmul(
            pv0[0:65, :],
            vaug[:, k, 2 * p, :],
            ex[:, 0:512],
            start=(k == 0),
            stop=(k == TT - 1),
        )
        nc.tensor.matmul(
            pv1[0:65, :],
            vaug[:, k, 2 * p + 1, :],
            ex[:, 512:1024],
            start=(k == 0),
            stop=(k == TT - 1),
        )

    def normalize_a(pv):
        """Prompt DVE part: copy PSUM accumulator out, reciprocal of denoms."""
        pvs = pvsp.tile([65, 512], F32, tag="pvs")
        nc.vector.tensor_copy(pvs[:], pv[0:65, :])
        rc = rcp.tile([65, 512], F32R, tag="rc")
        lnd = rcp.tile([65, 512], F32, tag="lnd")
        nc.scalar.activation(lnd[64:65, :], pvs[64:65, :], AF.Ln)
        nc.scalar.activation(rc[64:65, :], lnd[64:65, :], AF.Exp, scale=-1.0)
        return pvs, rc

    def normalize_b(p, qc, h, pvs, rc):
        """Deferred PE+DVE part: broadcast recip, scale, store to apn."""
        bc = ps_sc.tile([128, 1024], F32, tag="sc")
        nc.tensor.matmul(
            bc[0:64, 0:512], ones_bc[64:65, :], rc[64:65, :], start=True, stop=True
        )
        if h % 2 == 0:
            nc.vector.tensor_tensor(
                apn[0:64, p, qc * 512 : (qc + 1) * 512],
                pvs[0:64, :],
                bc[0:64, 0:512],
                ALU.mult,
            )
        else:
            st = stp.tile([64, 512], F32R, tag="st")
            nc.vector.tensor_tensor(st[:], pvs[0:64, :], bc[0:64, 0:512], ALU.mult)
            nc.sync.dma_start(apn[64:128, p, qc * 512 : (qc + 1) * 512], st[:])

    # ---------- filler schedule ----------
    # pair 0 needs, before global step s = qc*TT + k:
    #   V(tt=k) for attnV, KT0 chunk k//4 for scores, QT0 chunk qc.
    # Prologue: V(0), KT0c0, QT0c0.  Due-driven emission handles the rest;
    # extra (non-due) filler is paced evenly across the remaining steps.
    v_chunk(0)
    qkt_chunk(kt, wk, pk_d, 0, 0, "pkc")
    qkt_chunk(qt, wq, pq_d, 0, 0, "pqc")

    due = []  # (due_step, emit_fn) for pair 0; due_step = qc*TT + k
    for ttc in range(1, TT):
        due.append((ttc, lambda ttc=ttc: v_chunk(ttc)))
    for j in range(1, QC):
        due.append((4 * j, lambda j=j: qkt_chunk(kt, wk, pk_d, 0, j, "pkc")))
        due.append(((j - 1) * TT + 10, lambda j=j: qkt_chunk(qt, wq, pq_d, 0, j, "pqc")))
    pace = [  # no hard deadline within pair 0: ft=1 projections
        (lambda j=j: qkt_chunk(kt, wk, pk_d, 1, j, "pkc")) for j in range(QC)
    ] + [
        (lambda j=j: qkt_chunk(qt, wq, pq_d, 1, j, "pqc")) for j in range(QC)
    ]
    due.sort(key=lambda t: t[0])
    due_i = 0
    pace_i = 0
    norm_q = []
    for qc in range(QC):
        pv0 = ps_mm.tile([128, 512], F32, tag="mm")
        pv1 = ps_mm.tile([128, 512], F32, tag="mm")
        for k in range(TT):
            s = qc * TT + k
            while due_i < len(due) and due[due_i][0] <= s:
                due[due_i][1]()
                due_i += 1
            if k in (2, 4) and norm_q:
                norm_q.pop(0)()
            if s % 7 == 6 and pace_i < len(pace):
                pace[pace_i]()
                pace_i += 1
            attn_step(0, qc, k, pv0, pv1)
        a0 = normalize_a(pv0)
        a1 = normalize_a(pv1)
        norm_q.append(lambda qc=qc, a=a0: normalize_b(0, qc, 0, *a))
        norm_q.append(lambda qc=qc, a=a1: normalize_b(0, qc, 1, *a))
    while pace_i < len(pace):
        pace[pace_i]()
        pace_i += 1
    for f in norm_q:
        f()
    norm_q = []

    # pair 1: interleave out-projection chunks for finished t-chunks.
    # outproj(:, tcc) needs apn ft0 (done) and ft1 at tcc -> after qc=tcc.
    op_queue = []
    for qc in range(QC):
        pv0 = ps_mm.tile([128, 512], F32, tag="mm")
        pv1 = ps_mm.tile([128, 512], F32, tag="mm")
        for k in range(TT):
            if k in (2, 4) and norm_q:
                norm_q.pop(0)()
            if k >= 5 and k % 2 == 1 and op_queue:
                op_queue.pop(0)()
            attn_step(1, qc, k, pv0, pv1)
        a0 = normalize_a(pv0)
        a1 = normalize_a(pv1)
        norm_q.append(lambda qc=qc, a=a0: normalize_b(1, qc, 0, *a))
        norm_q.append(lambda qc=qc, a=a1: normalize_b(1, qc, 1, *a))
        if qc > 0:
            op_queue.extend(
                [lambda mt=mt, qc=qc: outproj_chunk(mt, qc - 1) for mt in range(DT)]
            )
    for f in norm_q:
        f()
    for f in op_queue:
        f()
    for mt in range(DT):
        outproj_chunk(mt, QC - 1)


_NC = {}


def _get_nc(iters=1):
    if iters not in _NC:
        _NC[iters] = build_nc(iters)
    return _NC[iters]


def _sinusoid_pe():
    pos = np.arange(MAX_SEQ_LEN, dtype=np.float32)[:, None]
    div = np.exp(
        np.arange(0, D, 2, dtype=np.float32) * np.float32(-np.log(10000.0) / D)
    )
    ang = pos * div[None, :]
    pe = np.stack([np.sin(ang), np.cos(ang)], axis=-1).reshape(MAX_SEQ_LEN, D)
    return pe.astype(np.float32)


def make_in_maps(x, rel_emb, alpha, Wq, bq, Wk, bk, Wv, bv, Wo, bo):
    alpha = np.float32(alpha)
    abs_pe = _sinusoid_pe()[:S]
    rel_pe = rel_emb[MAX_SEQ_LEN - S : MAX_SEQ_LEN]
    pe = (alpha * abs_pe + (np.float32(1.0) - alpha) * rel_pe).astype(np.float32)

    in_maps = []
    for c in range(NCORES):
        b, g = divmod(c, 4)
        fsl = slice(g * F, (g + 1) * F)
        wq_s, wk_s, wv_s = Wq[fsl], Wk[fsl], Wv[fsl]
        in_maps.append(
            {
                "xt": np.ascontiguousarray(x[b].T).astype(ml_dtypes.bfloat16),
                "wqt": np.ascontiguousarray(wq_s.T).astype(ml_dtypes.bfloat16),
                "wkt": np.ascontiguousarray(wk_s.T).astype(ml_dtypes.bfloat16),
                "wvt": np.ascontiguousarray(wv_s.T).astype(ml_dtypes.bfloat16),
                "wot": np.ascontiguousarray(Wo[:, fsl].T),
                "pq": np.ascontiguousarray((pe @ wq_s.T + bq[fsl]).T),
                "pk": np.ascontiguousarray((pe @ wk_s.T + bk[fsl]).T),
                "pv": np.ascontiguousarray(pe @ wv_s.T + bv[fsl]),
            }
        )
    return in_maps


def unshard(results, bo):
    y = np.empty((B, S, D), dtype=np.float32)
    for b in range(B):
        acc = results[4 * b]["yt"].astype(np.float32).copy()
        for g in range(1, 4):
            acc += results[4 * b + g]["yt"]
        y[b] = acc.T + bo
    return y


def kernel(x, rel_emb, alpha, Wq, bq, Wk, bk, Wv, bv, Wo, bo, **kw):
    x = np.asarray(x, dtype=np.float32)
    args = [
        np.asarray(a, dtype=np.float32)
        for a in (rel_emb, alpha, Wq, bq, Wk, bk, Wv, bv, Wo, bo)
    ]
    nc = _get_nc()
    in_maps = make_in_maps(x, *args)
    res = run_bass_kernel_spmd(nc, in_maps, core_ids=list(range(NCORES)))
    return unshard(res.results, args[-1])
